# revision 18
# baseline (speedup 1.0000x reference)
"""Trainium2 Bass kernel for nn_MoELayer_15934328668398 (moe_routing).

MoE layer: B=4, T=1024, D=2048, F=1024, E=8 experts, top-2 routing.

Math note: the reference's dispatch mask is redundant — combine_weights
already zero out unselected experts and the FFN is pointwise per token, so
    out[t] = sum_e w_e[t] * FFN_e(x[t])
with w_e[t] = renormalized top-2 softmax weight (0 if e not in top-2).

Strategy (expert-parallel dispatch, two launches):
  1. Router launch: tokens sharded 512/core; each core computes fp32
     scores + top-2 renormalized softmax weights for its tokens.
  2. Host dispatch (index shuffling only): bucket token ids by expert.
  3. FFN launch: core c gets expert c's weights plus its <=1152 gathered
     tokens (pre-transposed); computes silu(xWg) * (xWu) @ Wd scaled by the
     combine weight, in float32r (full PE rate at free-dim >= 256).
  4. Host unshard: scatter-add the two weighted expert outputs per token.

Capacity C=1152 covers the observed per-expert load (~1030 +- 30) with
margin; if any expert ever exceeds it, we fall back to a dense
token-sharded kernel (every core: 512 tokens x all 8 experts) that is
always correct.

Precision: router matmul in fp32 (top-2 selection is sensitive to score
noise; min #2-#3 score gap is ~3e-4); FFN matmuls in float32r (~3e-4 rel
err overall).
"""

import numpy as np

import concourse.mybir as mybir
import concourse.tile as tile
from concourse import bacc
from concourse.bass_utils import run_bass_kernel_spmd

B, T, D, F, E = 4, 1024, 2048, 1024, 8
NCORES = 8
NTOK = B * T              # 4096 tokens
TOK = NTOK // NCORES      # 512 tokens per core (router / dense sharding)
P = 128
KD = D // P               # 16 k-tiles contracting D
MF = F // P               # 8 f-tiles (partition tiles of F)
MT = TOK // P             # 4 token m-tiles (router / dense)
NBLK = 512                # fp32r-friendly free-dim block
CAP = 1152                # per-expert token capacity (multiple of 384)
CB = 384                  # token block in gate/up matmuls (>=256 for fp32r)
CM = CAP // P             # 9 token m-tiles in the down matmul
F32 = mybir.dt.float32
F32R = mybir.dt.float32r
EXP = mybir.ActivationFunctionType.Exp
SILU = mybir.ActivationFunctionType.Silu

_CACHE = {}
LAST_RESULTS = {}


def _topk_block(nc, sm, s, w8, m):
    """Emit top2->renormalized-weights from scores tile s [P, E] (f32)."""
    mx = sm.tile([P, 8], F32, name="mx")
    nc.vector.max(mx[:], s[:])
    negm1 = sm.tile([P, 1], F32, name="negm1")
    nc.vector.tensor_scalar_mul(negm1[:], mx[:, 0:1], -1.0)
    e2 = sm.tile([P, 1], F32, name="e2")
    nc.scalar.activation(e2[:], mx[:, 1:2], EXP, bias=negm1[:])
    den = sm.tile([P, 1], F32, name="den")
    nc.vector.tensor_scalar_add(den[:], e2[:], 1.0)
    rec = sm.tile([P, 1], F32, name="rec")
    nc.vector.reciprocal(rec[:], den[:])
    es = sm.tile([P, E], F32, name="es")
    nc.scalar.activation(es[:], s[:], EXP, bias=negm1[:])
    msk = sm.tile([P, E], F32, name="msk")
    nc.vector.tensor_scalar(msk[:], s[:], mx[:, 1:2], None,
                            op0=mybir.AluOpType.is_ge)
    wa = sm.tile([P, E], F32, name="wa")
    nc.vector.tensor_scalar_mul(wa[:], es[:], rec[:])
    nc.vector.tensor_mul(w8[:, m, :], wa[:], msk[:])


def _build_router():
    """Launch 1: 512 tokens/core -> [512, 8] combine weights.

    Scores are computed transposed ([E, TOK] with the 8-column router weight
    stationary, N=512 moving) in 16 fp32 matmuls, then PE-transposed back to
    token-major [P, E] tiles for the free-dim top-2 math.
    """
    from concourse.masks import make_identity

    nc = bacc.Bacc("TRN2", target_bir_lowering=False, debug=False,
                   num_devices=NCORES)
    xT = nc.dram_tensor("xT", [P, KD, TOK], F32, kind="ExternalInput").ap()
    rw = nc.dram_tensor("rw", [P, KD, E], F32, kind="ExternalInput").ap()
    w8o = nc.dram_tensor("w8", [TOK, E], F32, kind="ExternalOutput").ap()

    with tile.TileContext(nc) as tc:
        with tc.tile_pool(name="big", bufs=1) as big, \
             tc.tile_pool(name="sm", bufs=2) as sm, \
             tc.tile_pool(name="pst", bufs=1, space="PSUM") as pst, \
             tc.tile_pool(name="psr", bufs=2, space="PSUM") as psr:
            rw_sb = big.tile([P, KD, E], F32, name="rw_sb")
            nc.sync.dma_start(rw_sb[:], rw)
            xT_sb = big.tile([P, KD, TOK], F32, name="xT_sb")
            for k in range(KD):
                eng = nc.sync if k % 2 == 0 else nc.scalar
                eng.dma_start(xT_sb[:, k, :], xT[:, k, :])
            ident = big.tile([P, P], F32, name="ident")
            make_identity(nc, ident)
            w8 = big.tile([P, MT, E], F32, name="w8")

            ps_sT = pst.tile([E, TOK], F32, name="ps_sT")
            for k in range(KD):
                nc.tensor.matmul(ps_sT[:], rw_sb[:, k, :], xT_sb[:, k, :],
                                 start=(k == 0), stop=(k == KD - 1))
            sT = big.tile([E, TOK], F32, name="sT")
            nc.vector.tensor_copy(sT[:], ps_sT[:])
            for m in range(MT):
                ps_t = psr.tile([P, E], F32, name="ps_t")
                nc.tensor.transpose(ps_t[:], sT[:, m * P:(m + 1) * P],
                                    ident[:E, :E])
                s = sm.tile([P, E], F32, name="s")
                nc.vector.tensor_copy(s[:], ps_t[:])
                _topk_block(nc, sm, s, w8, m)
            nc.sync.dma_start(
                w8o.rearrange("(m p) e -> p m e", p=P), w8[:])
    nc.compile()
    return nc


def _build_ffn():
    """Launch 2: one expert/core, FFN over CAP gathered tokens."""
    nc = bacc.Bacc("TRN2", target_bir_lowering=False, debug=False,
                   num_devices=NCORES)
    DM = D // P   # 16 down-output tiles
    xTg = nc.dram_tensor("xTg", [P, KD, CAP], F32, kind="ExternalInput").ap()
    gw = nc.dram_tensor("gw", [MF, P, KD, P], F32, kind="ExternalInput").ap()
    uw = nc.dram_tensor("uw", [MF, P, KD, P], F32, kind="ExternalInput").ap()
    dw = nc.dram_tensor("dw", [DM, P, MF, P], F32, kind="ExternalInput").ap()
    wv = nc.dram_tensor("wv", [P, CAP], F32, kind="ExternalInput").ap()
    ygT = nc.dram_tensor("ygT", [D, CAP], F32, kind="ExternalOutput").ap()

    with tile.TileContext(nc) as tc:
        with tc.tile_pool(name="big", bufs=1) as big, \
             tc.tile_pool(name="wg", bufs=2) as wgp, \
             tc.tile_pool(name="wu", bufs=2) as wup, \
             tc.tile_pool(name="wd", bufs=2) as wdp, \
             tc.tile_pool(name="sm", bufs=2) as sm, \
             tc.tile_pool(name="out", bufs=3) as outp, \
             tc.tile_pool(name="psg", bufs=3, space="PSUM") as psg, \
             tc.tile_pool(name="psu", bufs=3, space="PSUM") as psu, \
             tc.tile_pool(name="psy", bufs=2, space="PSUM") as psy:

            NCB = CAP // CB
            xTg_sb = big.tile([P, KD, CAP], F32R, name="xTg_sb")   # 9.4 MB

            def load_xtg(k):
                eng = nc.sync if k % 2 == 0 else nc.scalar
                eng.dma_start(xTg_sb[:, k, :], xTg[:, k, :].bitcast(F32R))

            # First f-pass dependencies first: xTg k=0,1 + f=0 weights, then
            # the rest of the stream (k-outer matmuls chase the k-tiles).
            load_xtg(0)
            load_xtg(1)
            wv_sb = big.tile([P, CAP], F32, name="wv_sb")
            nc.sync.dma_start(wv_sb[:], wv)
            aT = big.tile([P, MF, CAP], F32R, name="aT")           # 4.7 MB

            # Gate & up projections -> aT = silu(G^T) * U^T (f32r).
            for f in range(MF):
                wg_t = wgp.tile([P, KD, P], F32R, tag="wg", name="wg_t")
                wu_t = wup.tile([P, KD, P], F32R, tag="wu", name="wu_t")
                for kq in range(4):
                    ks = slice(kq * (KD // 4), (kq + 1) * (KD // 4))
                    nc.sync.dma_start(wg_t[:, ks, :], gw[f, :, ks, :].bitcast(F32R))
                    nc.scalar.dma_start(wu_t[:, ks, :], uw[f, :, ks, :].bitcast(F32R))
                if f == 0:
                    for k in range(2, KD):
                        load_xtg(k)
                ps_gs = [psg.tile([P, CB], F32, name="ps_g") for _ in range(NCB)]
                ps_us = [psu.tile([P, CB], F32, name="ps_u") for _ in range(NCB)]
                for k in range(KD):
                    for cb in range(NCB):
                        csl = slice(cb * CB, (cb + 1) * CB)
                        nc.tensor.matmul(ps_gs[cb][:], wg_t[:, k, :],
                                         xTg_sb[:, k, csl],
                                         start=(k == 0), stop=(k == KD - 1))
                        nc.tensor.matmul(ps_us[cb][:], wu_t[:, k, :],
                                         xTg_sb[:, k, csl],
                                         start=(k == 0), stop=(k == KD - 1))
                for cb in range(NCB):
                    csl = slice(cb * CB, (cb + 1) * CB)
                    sil = sm.tile([P, CB], F32, tag="sil", name="sil")
                    nc.scalar.activation(sil[:], ps_gs[cb][:], SILU)
                    nc.vector.tensor_mul(aT[:, f, csl], sil[:], ps_us[cb][:])

            # Down projection (Y^T orientation: tokens on the free dim, so
            # N=384 stays on the fast matmul path), scaled by the
            # pre-broadcast combine weights, streamed out transposed.
            for m in range(DM):
                wd_t = wdp.tile([P, MF, P], F32R, tag="wd", name="wd_t")
                eng = nc.sync if m % 2 == 0 else nc.scalar
                eng.dma_start(wd_t[:], dw[m].bitcast(F32R))
                for cb in range(NCB):
                    csl = slice(cb * CB, (cb + 1) * CB)
                    ps_y = psy.tile([P, CB], F32, name="ps_y")
                    for f2 in range(MF):
                        nc.tensor.matmul(
                            ps_y[:],
                            wd_t[:, f2, :],
                            aT[:, f2, csl],
                            start=(f2 == 0), stop=(f2 == MF - 1),
                        )
                    o = outp.tile([P, CB], F32, tag="o", name="o")
                    nc.vector.tensor_mul(o[:], ps_y[:], wv_sb[:, csl])
                    nc.sync.dma_start(ygT[m * P:(m + 1) * P, csl], o[:])
    nc.compile()
    return nc


def _build_dense():
    """Fallback: dense token-sharded kernel (512 tokens x all experts)."""
    nc = bacc.Bacc("TRN2", target_bir_lowering=False, debug=False,
                   num_devices=NCORES)
    xT = nc.dram_tensor("xT", [P, KD, TOK], F32, kind="ExternalInput").ap()
    rw = nc.dram_tensor("rw", [P, KD, E], F32, kind="ExternalInput").ap()
    gw = nc.dram_tensor("gw", [E, MF, P, KD, P], F32, kind="ExternalInput").ap()
    uw = nc.dram_tensor("uw", [E, MF, P, KD, P], F32, kind="ExternalInput").ap()
    dw = nc.dram_tensor("dw", [E, F, D], F32, kind="ExternalInput").ap()
    y = nc.dram_tensor("y", [TOK, D], F32, kind="ExternalOutput").ap()

    from concourse.masks import make_identity

    dw_r = dw.rearrange("e (g p) d -> e g p d", p=P)   # [E, MF, P, D]

    with tile.TileContext(nc) as tc:
        with tc.tile_pool(name="big", bufs=1) as big, \
             tc.tile_pool(name="wg", bufs=2) as wgp, \
             tc.tile_pool(name="wu", bufs=2) as wup, \
             tc.tile_pool(name="wd", bufs=2) as wdp, \
             tc.tile_pool(name="sm", bufs=2) as sm, \
             tc.tile_pool(name="psg", bufs=2, space="PSUM") as psg, \
             tc.tile_pool(name="psu", bufs=2, space="PSUM") as psu, \
             tc.tile_pool(name="psy", bufs=2, space="PSUM") as psy, \
             tc.tile_pool(name="psr", bufs=1, space="PSUM") as psr:

            xT_sb = big.tile([P, KD, TOK], F32R, name="xT_sb")      # 4 MB
            for k in range(KD):
                nc.sync.dma_start(xT_sb[:, k, :], xT[:, k, :].bitcast(F32R))
            rw_sb = big.tile([P, KD, E], F32, name="rw_sb")
            nc.sync.dma_start(rw_sb[:], rw)
            ident = big.tile([P, P], F32, name="ident")
            make_identity(nc, ident)
            y_acc = big.tile([P, MT, D], F32, name="y_acc")         # 4 MB
            a_sb = big.tile([P, MF, TOK], F32R, name="a_sb")        # 2 MB
            w8 = big.tile([P, MT, E], F32, name="w8")

            ps_sT = psr.tile([E, TOK], F32, name="ps_sT")
            for k in range(KD):
                nc.tensor.matmul(ps_sT[:], rw_sb[:, k, :],
                                 xT_sb[:, k, :].bitcast(F32),
                                 start=(k == 0), stop=(k == KD - 1))
            sT = big.tile([E, TOK], F32, name="sT")
            nc.vector.tensor_copy(sT[:], ps_sT[:])
            for m in range(MT):
                ps_t = psr.tile([P, E], F32, name="ps_t")
                nc.tensor.transpose(ps_t[:], sT[:, m * P:(m + 1) * P],
                                    ident[:E, :E])
                s = sm.tile([P, E], F32, name="s")
                nc.vector.tensor_copy(s[:], ps_t[:])
                _topk_block(nc, sm, s, w8, m)

            for e in range(E):
                for f in range(MF):
                    wg_t = wgp.tile([P, KD, P], F32R, tag="wg", name="wg_t")
                    nc.sync.dma_start(wg_t[:], gw[e, f].bitcast(F32R))
                    wu_t = wup.tile([P, KD, P], F32R, tag="wu", name="wu_t")
                    nc.sync.dma_start(wu_t[:], uw[e, f].bitcast(F32R))
                    ps_g = psg.tile([P, TOK], F32, name="ps_g")
                    ps_u = psu.tile([P, TOK], F32, name="ps_u")
                    for k in range(KD):
                        nc.tensor.matmul(ps_g[:], wg_t[:, k, :],
                                         xT_sb[:, k, :],
                                         start=(k == 0), stop=(k == KD - 1))
                    for k in range(KD):
                        nc.tensor.matmul(ps_u[:], wu_t[:, k, :],
                                         xT_sb[:, k, :],
                                         start=(k == 0), stop=(k == KD - 1))
                    sil = sm.tile([P, TOK], F32, tag="sil", name="sil")
                    nc.scalar.activation(sil[:], ps_g[:], SILU)
                    nc.vector.tensor_mul(a_sb[:, f, :], sil[:], ps_u[:])

                for nh in range(2):
                    wd_t = wdp.tile([P, MF, D // 2], F32R, tag="wd",
                                    name="wd_t")
                    nc.sync.dma_start(
                        wd_t[:],
                        dw_r[e, :, :, nh * (D // 2):(nh + 1) * (D // 2)]
                        .rearrange("g p d -> p g d").bitcast(F32R))
                    for m in range(MT):
                        for n2 in range(D // 2 // NBLK):
                            ps_y = psy.tile([P, NBLK], F32, name="ps_y")
                            for f2 in range(MF):
                                nc.tensor.matmul(
                                    ps_y[:],
                                    a_sb[:, f2, m * P:(m + 1) * P],
                                    wd_t[:, f2,
                                         n2 * NBLK:(n2 + 1) * NBLK],
                                    start=(f2 == 0), stop=(f2 == MF - 1),
                                )
                            ysl = y_acc[:, m,
                                        nh * (D // 2) + n2 * NBLK:
                                        nh * (D // 2) + (n2 + 1) * NBLK]
                            wsl = w8[:, m, e:e + 1]
                            if e == 0:
                                nc.vector.tensor_scalar_mul(
                                    ysl, ps_y[:], wsl)
                            else:
                                nc.vector.scalar_tensor_tensor(
                                    ysl, ps_y[:], wsl, ysl,
                                    op0=mybir.AluOpType.mult,
                                    op1=mybir.AluOpType.add)

            for m in range(MT):
                nc.sync.dma_start(y[m * P:(m + 1) * P, :], y_acc[:, m, :])

    nc.compile()
    return nc


def _get(name):
    if name not in _CACHE:
        _CACHE[name] = {"router": _build_router, "ffn": _build_ffn,
                        "dense": _build_dense}[name]()
    return _CACHE[name]


def _tile_w(w):
    # [E, D, F] -> [E, MF, P, KD, P]: each (e, f) block DMAs with one
    # contiguous 8KB line per partition.
    return np.ascontiguousarray(
        w.reshape(E, KD, P, MF, P).transpose(0, 3, 2, 1, 4))


def _tile_dw(w):
    # [E, F, D] -> [E, DM, P, MF, P]: down weights as [F-part, D-col] tiles
    # grouped per D-tile, one contiguous 4KB line per partition.
    return np.ascontiguousarray(
        w.reshape(E, MF, P, D // P, P).transpose(0, 3, 2, 1, 4))


def _tile_xT(xrows):
    # [ntok, D] -> [P, KD, ntok] transposed tiling, contiguous lines.
    n = xrows.shape[0]
    return np.ascontiguousarray(
        xrows.T.reshape(KD, P, n).transpose(1, 0, 2))


def _run_router(xf, router_w):
    nc = _get("router")
    rwt = np.ascontiguousarray(router_w.reshape(KD, P, E).transpose(1, 0, 2))
    in_maps = [{"xT": _tile_xT(xf[c * TOK:(c + 1) * TOK]), "rw": rwt}
               for c in range(NCORES)]
    res = run_bass_kernel_spmd(nc, in_maps, core_ids=list(range(NCORES)))
    LAST_RESULTS["router"] = res
    return np.concatenate([res.results[c]["w8"] for c in range(NCORES)])


def _run_dense(xf, router_w, gate_proj, up_proj, down_proj):
    nc = _get("dense")
    gwt = _tile_w(np.ascontiguousarray(gate_proj))
    uwt = _tile_w(np.ascontiguousarray(up_proj))
    dwc = np.ascontiguousarray(down_proj)
    rwt = np.ascontiguousarray(router_w.reshape(KD, P, E).transpose(1, 0, 2))
    in_maps = []
    for c in range(NCORES):
        in_maps.append({"xT": _tile_xT(xf[c * TOK:(c + 1) * TOK]),
                        "rw": rwt, "gw": gwt, "uw": uwt, "dw": dwc})
    res = run_bass_kernel_spmd(nc, in_maps, core_ids=list(range(NCORES)))
    LAST_RESULTS["dense"] = res
    return np.concatenate([res.results[c]["y"] for c in range(NCORES)])


def kernel(x, router_w, gate_proj, up_proj, down_proj):
    global LAST_RESULTS
    LAST_RESULTS = {}
    x = np.ascontiguousarray(np.asarray(x, dtype=np.float32))
    router_w = np.asarray(router_w, dtype=np.float32)
    gate_proj = np.asarray(gate_proj, dtype=np.float32)
    up_proj = np.asarray(up_proj, dtype=np.float32)
    down_proj = np.asarray(down_proj, dtype=np.float32)
    xf = x.reshape(NTOK, D)

    # Launch 1: routing weights for every token (device-computed).
    w8_all = _run_router(xf, router_w)          # [NTOK, E]

    # Host dispatch: bucket token ids by expert (index work only).
    idxs = [np.nonzero(w8_all[:, e] > 0)[0] for e in range(E)]
    counts = [len(ix) for ix in idxs]
    if max(counts) > CAP:
        # Extremely unbalanced routing: dense fallback (always correct).
        y = _run_dense(xf, router_w, gate_proj, up_proj, down_proj)
        return y.reshape(B, T, D).astype(np.float32)

    gwt = _tile_w(np.ascontiguousarray(gate_proj))
    uwt = _tile_w(np.ascontiguousarray(up_proj))
    dwt = _tile_dw(np.ascontiguousarray(down_proj))
    in_maps = []
    for e in range(E):
        ix = idxs[e]
        xg = np.zeros((CAP, D), dtype=np.float32)
        xg[:len(ix)] = xf[ix]
        wvec = np.zeros(CAP, dtype=np.float32)
        wvec[:len(ix)] = w8_all[ix, e]
        in_maps.append({
            "xTg": _tile_xT(xg),
            "gw": gwt[e], "uw": uwt[e], "dw": dwt[e],
            "wv": np.ascontiguousarray(
                np.broadcast_to(wvec, (P, CAP))),
        })

    nc = _get("ffn")
    res = run_bass_kernel_spmd(nc, in_maps, core_ids=list(range(NCORES)))
    LAST_RESULTS["ffn"] = res

    # Host unshard: scatter-add the weighted expert outputs.
    y = np.zeros((NTOK, D), dtype=np.float32)
    for e in range(E):
        ix = idxs[e]
        y[ix] += res.results[e]["ygT"].T[:len(ix)]
    return y.reshape(B, T, D).astype(np.float32)


# revision 22
# speedup vs baseline: 1.1450x; 1.1450x over previous
"""Trainium2 Bass kernel for nn_MoELayer_15934328668398 (moe_routing).

MoE layer: B=4, T=1024, D=2048, F=1024, E=8 experts, top-2 routing.

Math note: the reference's dispatch mask is redundant — combine_weights
already zero out unselected experts and the FFN is pointwise per token, so
    out[t] = sum_e w_e[t] * FFN_e(x[t])
with w_e[t] = renormalized top-2 softmax weight (0 if e not in top-2).

Strategy (expert-parallel dispatch, two launches):
  1. Router launch: tokens sharded 512/core; each core computes fp32
     scores + top-2 renormalized softmax weights for its tokens.
  2. Host dispatch (index shuffling only): bucket token ids by expert.
  3. FFN launch: core c gets expert c's weights plus its <=1152 gathered
     tokens (pre-transposed); computes silu(xWg) * (xWu) @ Wd scaled by the
     combine weight, in float32r (full PE rate at free-dim >= 256).
  4. Host unshard: scatter-add the two weighted expert outputs per token.

Capacity C=1152 covers the observed per-expert load (~1030 +- 30) with
margin; if any expert ever exceeds it, we fall back to a dense
token-sharded kernel (every core: 512 tokens x all 8 experts) that is
always correct.

Precision: router matmul in fp32 (top-2 selection is sensitive to score
noise; min #2-#3 score gap is ~3e-4); FFN matmuls in float32r (~3e-4 rel
err overall).
"""

import numpy as np

import concourse.mybir as mybir
import concourse.tile as tile
from concourse import bacc
from concourse.bass_utils import run_bass_kernel_spmd

B, T, D, F, E = 4, 1024, 2048, 1024, 8
NCORES = 8
NTOK = B * T              # 4096 tokens
TOK = NTOK // NCORES      # 512 tokens per core (router / dense sharding)
P = 128
KD = D // P               # 16 k-tiles contracting D
MF = F // P               # 8 f-tiles (partition tiles of F)
MT = TOK // P             # 4 token m-tiles (router / dense)
NBLK = 512                # fp32r-friendly free-dim block
CAP = 1152                # per-expert token capacity (multiple of 384)
CB = 384                  # token block in gate/up matmuls (>=256 for fp32r)
CM = CAP // P             # 9 token m-tiles in the down matmul
F32 = mybir.dt.float32
F32R = mybir.dt.float32r
EXP = mybir.ActivationFunctionType.Exp
SILU = mybir.ActivationFunctionType.Silu

_CACHE = {}
LAST_RESULTS = {}


def _topk_block(nc, sm, s, w8, m):
    """Emit top2->renormalized-weights from scores tile s [P, E] (f32)."""
    mx = sm.tile([P, 8], F32, name="mx")
    nc.vector.max(mx[:], s[:])
    negm1 = sm.tile([P, 1], F32, name="negm1")
    nc.vector.tensor_scalar_mul(negm1[:], mx[:, 0:1], -1.0)
    e2 = sm.tile([P, 1], F32, name="e2")
    nc.scalar.activation(e2[:], mx[:, 1:2], EXP, bias=negm1[:])
    den = sm.tile([P, 1], F32, name="den")
    nc.vector.tensor_scalar_add(den[:], e2[:], 1.0)
    rec = sm.tile([P, 1], F32, name="rec")
    nc.vector.reciprocal(rec[:], den[:])
    es = sm.tile([P, E], F32, name="es")
    nc.scalar.activation(es[:], s[:], EXP, bias=negm1[:])
    msk = sm.tile([P, E], F32, name="msk")
    nc.vector.tensor_scalar(msk[:], s[:], mx[:, 1:2], None,
                            op0=mybir.AluOpType.is_ge)
    wa = sm.tile([P, E], F32, name="wa")
    nc.vector.tensor_scalar_mul(wa[:], es[:], rec[:])
    nc.vector.tensor_mul(w8[:, m, :], wa[:], msk[:])


def _build_router():
    """Launch 1: 512 tokens/core -> [512, 8] combine weights.

    Scores are computed transposed ([E, TOK] with the 8-column router weight
    stationary, N=512 moving) in 16 fp32 matmuls, then PE-transposed back to
    token-major [P, E] tiles for the free-dim top-2 math.
    """
    from concourse.masks import make_identity

    nc = bacc.Bacc("TRN2", target_bir_lowering=False, debug=False,
                   num_devices=NCORES)
    xT = nc.dram_tensor("xT", [P, KD, TOK], F32, kind="ExternalInput").ap()
    rw = nc.dram_tensor("rw", [P, KD, E], F32, kind="ExternalInput").ap()
    w8o = nc.dram_tensor("w8", [TOK, E], F32, kind="ExternalOutput").ap()

    with tile.TileContext(nc) as tc:
        with tc.tile_pool(name="big", bufs=1) as big, \
             tc.tile_pool(name="sm", bufs=2) as sm, \
             tc.tile_pool(name="pst", bufs=1, space="PSUM") as pst, \
             tc.tile_pool(name="psr", bufs=2, space="PSUM") as psr:
            rw_sb = big.tile([P, KD, E], F32, name="rw_sb")
            nc.sync.dma_start(rw_sb[:], rw)
            xT_sb = big.tile([P, KD, TOK], F32, name="xT_sb")
            for k in range(KD):
                eng = nc.sync if k % 2 == 0 else nc.scalar
                eng.dma_start(xT_sb[:, k, :], xT[:, k, :])
            ident = big.tile([P, P], F32, name="ident")
            make_identity(nc, ident)
            w8 = big.tile([P, MT, E], F32, name="w8")

            ps_sT = pst.tile([E, TOK], F32, name="ps_sT")
            for k in range(KD):
                nc.tensor.matmul(ps_sT[:], rw_sb[:, k, :], xT_sb[:, k, :],
                                 start=(k == 0), stop=(k == KD - 1))
            sT = big.tile([E, TOK], F32, name="sT")
            nc.vector.tensor_copy(sT[:], ps_sT[:])
            for m in range(MT):
                ps_t = psr.tile([P, E], F32, name="ps_t")
                nc.tensor.transpose(ps_t[:], sT[:, m * P:(m + 1) * P],
                                    ident[:E, :E])
                s = sm.tile([P, E], F32, name="s")
                nc.vector.tensor_copy(s[:], ps_t[:])
                _topk_block(nc, sm, s, w8, m)
            nc.sync.dma_start(
                w8o.rearrange("(m p) e -> p m e", p=P), w8[:])
    nc.compile()
    return nc


def _build_ffn():
    """Launch 2: one expert/core, FFN over CAP gathered tokens."""
    nc = bacc.Bacc("TRN2", target_bir_lowering=False, debug=False,
                   num_devices=NCORES)
    xTg = nc.dram_tensor("xTg", [P, KD, CAP], F32, kind="ExternalInput").ap()
    gw = nc.dram_tensor("gw", [MF, P, KD, P], F32, kind="ExternalInput").ap()
    uw = nc.dram_tensor("uw", [MF, P, KD, P], F32, kind="ExternalInput").ap()
    dw = nc.dram_tensor("dw", [F, D], F32, kind="ExternalInput").ap()
    wv = nc.dram_tensor("wv", [P, CM], F32, kind="ExternalInput").ap()
    yg = nc.dram_tensor("yg", [CAP, D], F32, kind="ExternalOutput").ap()

    dw_r = dw.rearrange("(g p) d -> p g d", p=P)   # [P, MF, D]

    with tile.TileContext(nc) as tc:
        with tc.tile_pool(name="big", bufs=1) as big, \
             tc.tile_pool(name="wg", bufs=2) as wgp, \
             tc.tile_pool(name="wu", bufs=2) as wup, \
             tc.tile_pool(name="wd", bufs=2) as wdp, \
             tc.tile_pool(name="sm", bufs=2) as sm, \
             tc.tile_pool(name="out", bufs=3) as outp, \
             tc.tile_pool(name="psg", bufs=3, space="PSUM") as psg, \
             tc.tile_pool(name="psu", bufs=3, space="PSUM") as psu, \
             tc.tile_pool(name="psy", bufs=2, space="PSUM") as psy:

            NCB = CAP // CB
            xTg_sb = big.tile([P, KD, CAP], F32R, name="xTg_sb")   # 9.4 MB

            def load_xtg(k):
                eng = nc.sync if k % 2 == 0 else nc.scalar
                eng.dma_start(xTg_sb[:, k, :], xTg[:, k, :].bitcast(F32R))

            # First f-pass dependencies first: xTg k=0,1 + f=0 weights, then
            # the rest of the stream (k-outer matmuls chase the k-tiles).
            load_xtg(0)
            load_xtg(1)
            wv_sb = big.tile([P, CM], F32, name="wv_sb")
            nc.sync.dma_start(wv_sb[:], wv)
            aT = big.tile([P, MF, CAP], F32R, name="aT")           # 4.7 MB

            # Gate & up projections -> aT = silu(G^T) * U^T (f32r).
            for f in range(MF):
                wg_t = wgp.tile([P, KD, P], F32R, tag="wg", name="wg_t")
                wu_t = wup.tile([P, KD, P], F32R, tag="wu", name="wu_t")
                for kq in range(4):
                    ks = slice(kq * (KD // 4), (kq + 1) * (KD // 4))
                    nc.sync.dma_start(wg_t[:, ks, :], gw[f, :, ks, :].bitcast(F32R))
                    nc.scalar.dma_start(wu_t[:, ks, :], uw[f, :, ks, :].bitcast(F32R))
                if f == 0:
                    for k in range(2, KD):
                        load_xtg(k)
                ps_gs = [psg.tile([P, CB], F32, name="ps_g") for _ in range(NCB)]
                ps_us = [psu.tile([P, CB], F32, name="ps_u") for _ in range(NCB)]
                for k in range(KD):
                    for cb in range(NCB):
                        csl = slice(cb * CB, (cb + 1) * CB)
                        nc.tensor.matmul(ps_gs[cb][:], wg_t[:, k, :],
                                         xTg_sb[:, k, csl],
                                         start=(k == 0), stop=(k == KD - 1))
                        nc.tensor.matmul(ps_us[cb][:], wu_t[:, k, :],
                                         xTg_sb[:, k, csl],
                                         start=(k == 0), stop=(k == KD - 1))
                for cb in range(NCB):
                    csl = slice(cb * CB, (cb + 1) * CB)
                    sil = sm.tile([P, CB], F32, tag="sil", name="sil")
                    nc.scalar.activation(sil[:], ps_gs[cb][:], SILU)
                    nc.vector.tensor_mul(aT[:, f, csl], sil[:], ps_us[cb][:])

            # Down projection, scaled by combine weight, streamed out.
            for n in range(D // NBLK):
                wd_t = wdp.tile([P, MF, NBLK], F32R, tag="wd", name="wd_t")
                for f2 in range(MF):
                    eng = nc.sync if f2 % 2 == 0 else nc.scalar
                    eng.dma_start(
                        wd_t[:, f2, :],
                        dw_r[:, f2, n * NBLK:(n + 1) * NBLK].bitcast(F32R))
                for m in range(CM):
                    ps_y = psy.tile([P, NBLK], F32, name="ps_y")
                    for f2 in range(MF):
                        nc.tensor.matmul(
                            ps_y[:],
                            aT[:, f2, m * P:(m + 1) * P],
                            wd_t[:, f2, :],
                            start=(f2 == 0), stop=(f2 == MF - 1),
                        )
                    o = outp.tile([P, NBLK], F32, tag="o", name="o")
                    nc.vector.tensor_scalar_mul(o[:], ps_y[:],
                                                wv_sb[:, m:m + 1])
                    nc.sync.dma_start(
                        yg[m * P:(m + 1) * P, n * NBLK:(n + 1) * NBLK],
                        o[:])
    nc.compile()
    return nc


def _build_dense():
    """Fallback: dense token-sharded kernel (512 tokens x all experts)."""
    nc = bacc.Bacc("TRN2", target_bir_lowering=False, debug=False,
                   num_devices=NCORES)
    xT = nc.dram_tensor("xT", [P, KD, TOK], F32, kind="ExternalInput").ap()
    rw = nc.dram_tensor("rw", [P, KD, E], F32, kind="ExternalInput").ap()
    gw = nc.dram_tensor("gw", [E, MF, P, KD, P], F32, kind="ExternalInput").ap()
    uw = nc.dram_tensor("uw", [E, MF, P, KD, P], F32, kind="ExternalInput").ap()
    dw = nc.dram_tensor("dw", [E, F, D], F32, kind="ExternalInput").ap()
    y = nc.dram_tensor("y", [TOK, D], F32, kind="ExternalOutput").ap()

    from concourse.masks import make_identity

    dw_r = dw.rearrange("e (g p) d -> e g p d", p=P)   # [E, MF, P, D]

    with tile.TileContext(nc) as tc:
        with tc.tile_pool(name="big", bufs=1) as big, \
             tc.tile_pool(name="wg", bufs=2) as wgp, \
             tc.tile_pool(name="wu", bufs=2) as wup, \
             tc.tile_pool(name="wd", bufs=2) as wdp, \
             tc.tile_pool(name="sm", bufs=2) as sm, \
             tc.tile_pool(name="psg", bufs=2, space="PSUM") as psg, \
             tc.tile_pool(name="psu", bufs=2, space="PSUM") as psu, \
             tc.tile_pool(name="psy", bufs=2, space="PSUM") as psy, \
             tc.tile_pool(name="psr", bufs=1, space="PSUM") as psr:

            xT_sb = big.tile([P, KD, TOK], F32R, name="xT_sb")      # 4 MB
            for k in range(KD):
                nc.sync.dma_start(xT_sb[:, k, :], xT[:, k, :].bitcast(F32R))
            rw_sb = big.tile([P, KD, E], F32, name="rw_sb")
            nc.sync.dma_start(rw_sb[:], rw)
            ident = big.tile([P, P], F32, name="ident")
            make_identity(nc, ident)
            y_acc = big.tile([P, MT, D], F32, name="y_acc")         # 4 MB
            a_sb = big.tile([P, MF, TOK], F32R, name="a_sb")        # 2 MB
            w8 = big.tile([P, MT, E], F32, name="w8")

            ps_sT = psr.tile([E, TOK], F32, name="ps_sT")
            for k in range(KD):
                nc.tensor.matmul(ps_sT[:], rw_sb[:, k, :],
                                 xT_sb[:, k, :].bitcast(F32),
                                 start=(k == 0), stop=(k == KD - 1))
            sT = big.tile([E, TOK], F32, name="sT")
            nc.vector.tensor_copy(sT[:], ps_sT[:])
            for m in range(MT):
                ps_t = psr.tile([P, E], F32, name="ps_t")
                nc.tensor.transpose(ps_t[:], sT[:, m * P:(m + 1) * P],
                                    ident[:E, :E])
                s = sm.tile([P, E], F32, name="s")
                nc.vector.tensor_copy(s[:], ps_t[:])
                _topk_block(nc, sm, s, w8, m)

            for e in range(E):
                for f in range(MF):
                    wg_t = wgp.tile([P, KD, P], F32R, tag="wg", name="wg_t")
                    nc.sync.dma_start(wg_t[:], gw[e, f].bitcast(F32R))
                    wu_t = wup.tile([P, KD, P], F32R, tag="wu", name="wu_t")
                    nc.sync.dma_start(wu_t[:], uw[e, f].bitcast(F32R))
                    ps_g = psg.tile([P, TOK], F32, name="ps_g")
                    ps_u = psu.tile([P, TOK], F32, name="ps_u")
                    for k in range(KD):
                        nc.tensor.matmul(ps_g[:], wg_t[:, k, :],
                                         xT_sb[:, k, :],
                                         start=(k == 0), stop=(k == KD - 1))
                    for k in range(KD):
                        nc.tensor.matmul(ps_u[:], wu_t[:, k, :],
                                         xT_sb[:, k, :],
                                         start=(k == 0), stop=(k == KD - 1))
                    sil = sm.tile([P, TOK], F32, tag="sil", name="sil")
                    nc.scalar.activation(sil[:], ps_g[:], SILU)
                    nc.vector.tensor_mul(a_sb[:, f, :], sil[:], ps_u[:])

                for nh in range(2):
                    wd_t = wdp.tile([P, MF, D // 2], F32R, tag="wd",
                                    name="wd_t")
                    nc.sync.dma_start(
                        wd_t[:],
                        dw_r[e, :, :, nh * (D // 2):(nh + 1) * (D // 2)]
                        .rearrange("g p d -> p g d").bitcast(F32R))
                    for m in range(MT):
                        for n2 in range(D // 2 // NBLK):
                            ps_y = psy.tile([P, NBLK], F32, name="ps_y")
                            for f2 in range(MF):
                                nc.tensor.matmul(
                                    ps_y[:],
                                    a_sb[:, f2, m * P:(m + 1) * P],
                                    wd_t[:, f2,
                                         n2 * NBLK:(n2 + 1) * NBLK],
                                    start=(f2 == 0), stop=(f2 == MF - 1),
                                )
                            ysl = y_acc[:, m,
                                        nh * (D // 2) + n2 * NBLK:
                                        nh * (D // 2) + (n2 + 1) * NBLK]
                            wsl = w8[:, m, e:e + 1]
                            if e == 0:
                                nc.vector.tensor_scalar_mul(
                                    ysl, ps_y[:], wsl)
                            else:
                                nc.vector.scalar_tensor_tensor(
                                    ysl, ps_y[:], wsl, ysl,
                                    op0=mybir.AluOpType.mult,
                                    op1=mybir.AluOpType.add)

            for m in range(MT):
                nc.sync.dma_start(y[m * P:(m + 1) * P, :], y_acc[:, m, :])

    nc.compile()
    return nc


def _get(name):
    if name not in _CACHE:
        _CACHE[name] = {"router": _build_router, "ffn": _build_ffn,
                        "dense": _build_dense}[name]()
    return _CACHE[name]


def _tile_w(w):
    # [E, D, F] -> [E, MF, P, KD, P]: each (e, f) block DMAs with one
    # contiguous 8KB line per partition.
    return np.ascontiguousarray(
        w.reshape(E, KD, P, MF, P).transpose(0, 3, 2, 1, 4))


def _tile_dw(w):
    # [E, F, D] -> [E, DM, P, MF, P]: down weights as [F-part, D-col] tiles
    # grouped per D-tile, one contiguous 4KB line per partition.
    return np.ascontiguousarray(
        w.reshape(E, MF, P, D // P, P).transpose(0, 3, 2, 1, 4))


def _tile_xT(xrows):
    # [ntok, D] -> [P, KD, ntok] transposed tiling, contiguous lines.
    n = xrows.shape[0]
    return np.ascontiguousarray(
        xrows.T.reshape(KD, P, n).transpose(1, 0, 2))


def _run_router(xf, router_w):
    nc = _get("router")
    rwt = np.ascontiguousarray(router_w.reshape(KD, P, E).transpose(1, 0, 2))
    in_maps = [{"xT": _tile_xT(xf[c * TOK:(c + 1) * TOK]), "rw": rwt}
               for c in range(NCORES)]
    res = run_bass_kernel_spmd(nc, in_maps, core_ids=list(range(NCORES)))
    LAST_RESULTS["router"] = res
    return np.concatenate([res.results[c]["w8"] for c in range(NCORES)])


def _run_dense(xf, router_w, gate_proj, up_proj, down_proj):
    nc = _get("dense")
    gwt = _tile_w(np.ascontiguousarray(gate_proj))
    uwt = _tile_w(np.ascontiguousarray(up_proj))
    dwc = np.ascontiguousarray(down_proj)
    rwt = np.ascontiguousarray(router_w.reshape(KD, P, E).transpose(1, 0, 2))
    in_maps = []
    for c in range(NCORES):
        in_maps.append({"xT": _tile_xT(xf[c * TOK:(c + 1) * TOK]),
                        "rw": rwt, "gw": gwt, "uw": uwt, "dw": dwc})
    res = run_bass_kernel_spmd(nc, in_maps, core_ids=list(range(NCORES)))
    LAST_RESULTS["dense"] = res
    return np.concatenate([res.results[c]["y"] for c in range(NCORES)])


def kernel(x, router_w, gate_proj, up_proj, down_proj):
    global LAST_RESULTS
    LAST_RESULTS = {}
    x = np.ascontiguousarray(np.asarray(x, dtype=np.float32))
    router_w = np.asarray(router_w, dtype=np.float32)
    gate_proj = np.asarray(gate_proj, dtype=np.float32)
    up_proj = np.asarray(up_proj, dtype=np.float32)
    down_proj = np.asarray(down_proj, dtype=np.float32)
    xf = x.reshape(NTOK, D)

    # Launch 1: routing weights for every token (device-computed).
    w8_all = _run_router(xf, router_w)          # [NTOK, E]

    # Host dispatch: bucket token ids by expert (index work only).
    idxs = [np.nonzero(w8_all[:, e] > 0)[0] for e in range(E)]
    counts = [len(ix) for ix in idxs]
    if max(counts) > CAP:
        # Extremely unbalanced routing: dense fallback (always correct).
        y = _run_dense(xf, router_w, gate_proj, up_proj, down_proj)
        return y.reshape(B, T, D).astype(np.float32)

    gwt = _tile_w(np.ascontiguousarray(gate_proj))
    uwt = _tile_w(np.ascontiguousarray(up_proj))
    in_maps = []
    for e in range(E):
        ix = idxs[e]
        xg = np.zeros((CAP, D), dtype=np.float32)
        xg[:len(ix)] = xf[ix]
        wvec = np.zeros(CAP, dtype=np.float32)
        wvec[:len(ix)] = w8_all[ix, e]
        in_maps.append({
            "xTg": _tile_xT(xg),
            "gw": gwt[e], "uw": uwt[e],
            "dw": np.ascontiguousarray(down_proj[e]),
            "wv": np.ascontiguousarray(wvec.reshape(CM, P).T),
        })

    nc = _get("ffn")
    res = run_bass_kernel_spmd(nc, in_maps, core_ids=list(range(NCORES)))
    LAST_RESULTS["ffn"] = res

    # Host unshard: scatter-add the weighted expert outputs.
    y = np.zeros((NTOK, D), dtype=np.float32)
    for e in range(E):
        ix = idxs[e]
        y[ix] += res.results[e]["yg"][:len(ix)]
    return y.reshape(B, T, D).astype(np.float32)
